# revision 39
# baseline (speedup 1.0000x reference)
"""DeepAR (2-layer LSTM, H=512) Trainium2 Bass kernel, 8-core data-parallel.

Model (see reference): x = concat(x_cont, emb0[cat0], emb1[cat1]) [B,T,56]
  -> LSTM(512) -> LSTM(512) -> mu = h@Wmu+bmu ; sigma = softplus(h@Wsig+bsig)

Sharding: batch B=256 split across 8 cores (32 rows each); params replicated.

Per-core device program (all matmul operands bf16, psum fp32):
  - embeddings: e0 via per-128-row-tile indirect DMA gathers (multi-index
    indirect DMA corrupts SBUF on HW; each gather holds the GPSIMD Q7 ~1us),
    e1 via a one-hot matmul (CARD1=100 <= 128): onehot[k,n] = (cat1[n]==k)
    built with one DVE is_equal against a replicated index row. Tiles are
    assembled with x_cont + a ones row and PE-transposed into x^T bf16, one
    SBUF tile per 128 (t,b)-columns (deps are tile-granular), with the
    per-tile work emitted interleaved into the scan loop (lookahead PRO) so
    the in-order PE queue never stalls on a not-yet-gathered tile. Weight
    DMAs are chunked so gather transfers interleave on the serial DMA engine.
  - fused transposed-gates scan: gates are computed TRANSPOSED as 16 chunks
    [128 gate dims, 32 batch] with the weight chunk as the PE stationary and
    h^T [128, 32] moving, so each matmul streams only 32 output rows (vs 512
    with batch on the partition dim) and h^T needs no per-step transpose.
    Gate columns are permuted [i, f, o, g]; the gates psum is split into
    three tiles (i|f, o, g) so each activation waits only its own chunks.
    L2 runs TWO steps behind L1 in the same loop: every matmul of a block is
    ready when the block starts, so the in-order PE queue never waits on the
    activation tail. b2 is seeded with one K=16 one-hot matmul per gate tile.
  - head: mu/sigma^T [1, 128] = sum_c WmsT_c @ h2T_hist every 4 steps;
    mu += bmu (DVE); raw sigma pre-activations are staged in SBUF and
    softplus = Ln(Exp(x + bsig) + 1) runs ONCE batched after the scan
    (Exp/Ln live in a different ACT table set than Sigmoid/Tanh, so inline
    use would cost two 1.3us table swaps every head slice).
"""

import numpy as np
import ml_dtypes

import concourse.bass as bass
import concourse.mybir as mybir
import concourse.tile as tile
from concourse import bacc
from concourse.masks import make_identity

F32 = mybir.dt.float32
BF16 = mybir.dt.bfloat16
I32 = mybir.dt.int32

B, T, F = 256, 192, 8
CARD0, CARD1 = 1000, 100
E0, E1 = 32, 16
H = 512
DIN = F + E0 + E1          # 56
G4 = 4 * H                 # 2048
NC_N = 8                   # cores
BSH = B // NC_N            # 32 batch rows per core
R = T * BSH                # 6144 (t,b)-ordered rows per core
KC = H // 128              # 4 recurrent K-chunks
NM = G4 // 128             # 16 gate-dim chunks
A = mybir.ActivationFunctionType

# chunk emission order: i,f first (sig(i|f) starts the critical chain),
# then g (tanh(g) ready right before i*g), then o (only needed at the end)
_M_ORDER = [12, 13, 14, 15, 0, 1, 2, 3, 4, 5, 6, 7, 8, 9, 10, 11]
# L2 uses merged sig(i|f|o), so its i,f,o chunks go first (ready mid-run);
# g last feeds only the DVE i*g product, which L2's 2-step slack absorbs
_M_ORDER2 = [0, 1, 2, 3, 4, 5, 6, 7, 8, 9, 10, 11, 12, 13, 14, 15]

_NC_CACHE = {}


def build_nc(upto="all"):
    if upto in _NC_CACHE:
        return _NC_CACHE[upto]
    nc = bacc.Bacc("TRN2", num_devices=NC_N)

    # ---------------- DRAM I/O ----------------
    idx0_d = nc.dram_tensor("idx0", [128, R // 128], I32, kind="ExternalInput")
    idx1r_d = nc.dram_tensor("idx1r", [1, R], BF16, kind="ExternalInput")
    iota1_d = nc.dram_tensor("iota1", [CARD1, 1], F32, kind="ExternalInput")
    e0t_d = nc.dram_tensor("e0tab", [CARD0, E0], F32, kind="ExternalInput")
    e1t_d = nc.dram_tensor("e1t16", [CARD1, E1], BF16, kind="ExternalInput")
    xcr_d = nc.dram_tensor("xcr", [128, R // 128, F], F32, kind="ExternalInput")
    w1x_d = nc.dram_tensor("w1x", [64, G4], BF16, kind="ExternalInput")
    wr1_d = nc.dram_tensor("wr1", [128, KC, G4], BF16, kind="ExternalInput")
    wk2_d = nc.dram_tensor("wk2", [128, KC, G4], BF16, kind="ExternalInput")
    wr2_d = nc.dram_tensor("wr2", [128, KC, G4], BF16, kind="ExternalInput")
    b2t_d = nc.dram_tensor("b2t", [16, 128], BF16, kind="ExternalInput")
    oneh_d = nc.dram_tensor("oneh", [16, 512], BF16, kind="ExternalInput")
    wms_d = nc.dram_tensor("wms", [128, KC, 2], BF16, kind="ExternalInput")
    bms_d = nc.dram_tensor("bms", [1, 2], F32, kind="ExternalInput")

    mu_d = nc.dram_tensor("mu", [BSH, T], F32, kind="ExternalOutput")
    sg_d = nc.dram_tensor("sigma", [BSH, T], F32, kind="ExternalOutput")
    dbg_d = nc.dram_tensor("dbg", [64, R], F32, kind="ExternalOutput") \
        if upto != "all" else None

    _build_body(nc, upto, locals())
    nc.compile()
    _NC_CACHE[upto] = nc
    return nc


def _build_body(nc, upto, env):
    from contextlib import ExitStack
    idx0_d = env["idx0_d"]; idx1r_d = env["idx1r_d"]; xcr_d = env["xcr_d"]
    iota1_d = env["iota1_d"]
    e0t_d = env["e0t_d"]; e1t_d = env["e1t_d"]
    w1x_d = env["w1x_d"]; wr1_d = env["wr1_d"]
    wk2_d = env["wk2_d"]; wr2_d = env["wr2_d"]
    b2t_d = env["b2t_d"]; oneh_d = env["oneh_d"]
    wms_d = env["wms_d"]; bms_d = env["bms_d"]; mu_d = env["mu_d"]
    sg_d = env["sg_d"]; dbg_d = env["dbg_d"]
    MT = R // 128
    TSL = 4  # timesteps per head slice (small slices avoid a PE spike)

    with tile.TileContext(nc) as tc, ExitStack() as top:  # noqa: SIM117
        singles = top.enter_context(tc.tile_pool(name="singles", bufs=1))
        # scan pools are opened BEFORE the phase-1 pools so they never share
        # PSUM banks / SBUF ranges with them (sharing would add WAR waits
        # serializing scan start behind the last phase-1 gather/transpose)
        ew = top.enter_context(tc.tile_pool(name="ew", bufs=8))
        pg1p = top.enter_context(tc.tile_pool(name="pg1", bufs=1, space="PSUM"))
        pg2p = top.enter_context(tc.tile_pool(name="pg2", bufs=1, space="PSUM"))
        psh = top.enter_context(tc.tile_pool(name="psh", bufs=1, space="PSUM"))
        hew = top.enter_context(tc.tile_pool(name="hew", bufs=2))

        # ---------------- constants / weights to SBUF ----------------
        # small input tensors first so they don't queue behind 6.3MB of
        # weights on the DMA ring (phase 1 needs them immediately)
        singles_idx = singles
        idx0_sb = singles_idx.tile([128, MT], I32)
        nc.sync.dma_start(out=idx0_sb[:], in_=idx0_d[:])
        xcb = singles.tile([128, MT, F], F32)
        nc.sync.dma_start(out=xcb[:], in_=xcr_d[:])
        iota1_sb = singles.tile([CARD1, 1], F32)
        nc.sync.dma_start(out=iota1_sb[:], in_=iota1_d[:])
        e1t_sb = singles.tile([CARD1, E1], BF16)
        nc.sync.dma_start(out=e1t_sb[:], in_=e1t_d[:])
        idx1rep = singles.tile([CARD1, R], BF16)
        idx1_rep_src = bass.AP(tensor=idx1r_d[:].tensor, offset=0,
                               ap=[[0, CARD1], [1, R]])
        nc.sync.dma_start(out=idx1rep[:], in_=idx1_rep_src)
        w1x_sb = singles.tile([64, G4], BF16)
        nc.sync.dma_start(out=w1x_sb[:], in_=w1x_d[:])
        # big weight loads in per-chunk pieces so the short per-tile gather
        # transfers can interleave on the (serial) DMA engine
        wr1_sb = singles.tile([128, KC, G4], BF16)
        for c in range(KC):
            nc.sync.dma_start(out=wr1_sb[:, c, :], in_=wr1_d[:, c, :])
        b2t_sb = singles.tile([16, 128], BF16)
        nc.sync.dma_start(out=b2t_sb[:], in_=b2t_d[:])
        oneh_sb = singles.tile([16, 512], BF16)
        nc.sync.dma_start(out=oneh_sb[:], in_=oneh_d[:])
        wms_sb = singles.tile([128, KC, 2], BF16)
        nc.sync.dma_start(out=wms_sb[:], in_=wms_d[:])
        bms_sb = singles.tile([1, 2], F32)
        nc.sync.dma_start(out=bms_sb[:], in_=bms_d[:])
        wk2_sb = singles.tile([128, KC, G4], BF16)
        for c in range(KC):
            nc.sync.dma_start(out=wk2_sb[:, c, :], in_=wk2_d[:, c, :])
        wr2_sb = singles.tile([128, KC, G4], BF16)
        for c in range(KC):
            nc.sync.dma_start(out=wr2_sb[:, c, :], in_=wr2_d[:, c, :])

        ident_f32 = singles.tile([128, 128], F32)
        make_identity(nc, ident_f32[:])

        # x^T as one tile PER 128-column block: dependencies are tracked at
        # tile granularity, so a single xT tile would make scan step 0 wait
        # for the LAST phase-1 gather (~100us of serial indirect DMAs)
        xTs = [singles.tile([64, 128], BF16, name=f"xT{m}", tag=f"xT{m}")
               for m in range(MT)]
        h1T = singles.tile([128, 4, KC, BSH], BF16)   # 4-deep (L2 lags 2 steps)
        h2T = singles.tile([128, KC, T, BSH], BF16)   # full history (head)
        c1 = singles.tile([128, 128], BF16)
        c2 = singles.tile([128, 128], BF16)
        sgacc = singles.tile([1, T * BSH], F32)   # raw sigma pre-activations
        exp_all = singles.tile([1, T * BSH], BF16)
        nc.vector.memset(c1[:], 0.0)
        nc.vector.memset(c2[:], 0.0)

        # ---------------- phase 1: build x^T ----------------
        # The per-tile work is emitted INTERLEAVED into the scan loop with an
        # 8-tile lookahead: the in-order PE queue otherwise places each
        # transpose far ahead of its gather's completion and every early scan
        # step stalls on the (1us-per-gather) software-DGE pipeline.
        gp = top.enter_context(tc.tile_pool(name="gather", bufs=1))
        ptr = top.enter_context(tc.tile_pool(name="gtr", bufs=1, space="PSUM"))
        # e1 lookup as a one-hot matmul (CARD1=100 <= 128): cheaper than
        # 48 more 1us software-DGE gathers on the Pool engine
        onehot1 = gp.tile([CARD1, R], BF16)
        nc.vector.tensor_scalar(onehot1[:], idx1rep[:], iota1_sb[:],
                                None, op0=mybir.AluOpType.is_equal)

        def emit_tile(m):
            # assembled rows: [p, 64] = [e0 | e1(pad) | xc | ones]
            # NOTE: multi-index indirect DMA is broken on HW (stomps memory);
            # one gather per 128-row tile, single idx column each.
            asm = gp.tile([128, 64], F32, name=f"asm{m}", tag=f"asm{m}")
            nc.vector.memset(asm[:], 1.0)
            nc.gpsimd.tensor_copy(asm[:, E0 + E1:DIN], xcb[:, m, :])
            nc.gpsimd.indirect_dma_start(
                out=asm[:, 0:E0], out_offset=None, in_=e0t_d[:],
                in_offset=bass.IndirectOffsetOnAxis(
                    ap=idx0_sb[:, m:m + 1], axis=0))
            ps = ptr.tile([80, 128], F32, name=f"ps{m}", tag="ps")
            nc.tensor.transpose(ps[0:64, :], asm[:], ident_f32[:])
            nc.vector.tensor_copy(xTs[m][:], ps[0:64, :])
            nc.tensor.matmul(ps[64:80, :], e1t_sb[:],
                             onehot1[:, 128 * m:128 * (m + 1)],
                             start=True, stop=True)
            nc.vector.tensor_copy(xTs[m][E0:E0 + E1, :], ps[64:80, :])

        PRO = 4   # tiles emitted before the scan starts (supply lookahead)

        if upto == "xT":
            for m in range(MT):
                emit_tile(m)
            with tc.tile_pool(name="dbgp", bufs=1) as dp:
                dbg_sb = dp.tile([64, R], F32)
                for m in range(MT):
                    nc.vector.tensor_copy(
                        dbg_sb[:, 128 * m:128 * (m + 1)], xTs[m][:])
                nc.sync.dma_start(out=dbg_d[:], in_=dbg_sb[:])
            return
        for m in range(PRO):
            emit_tile(m)

        # -------- phase 2: fused transposed-gates scan (L2 two steps behind,
        # so every matmul of a block is ready when the block's run starts and
        # L2's psum accumulation group never blocks L1 on the in-order PE
        # queue) --------
        if True:
            # gates psum is split into three tiles per step — (i|f), (o), (g) —
            # because RAW deps are tile-granular: one [128,512] tile would make
            # every gate activation wait for ALL 80 matmuls of the step.
            def gate_slot(pool, tagp, merged):
                if merged:   # (i|f|o) in one tile, g separate
                    return (pool.tile([128, 384], F32, name=tagp + "ifo",
                                      tag=tagp + "ifo"),
                            None,
                            pool.tile([128, 128], F32, name=tagp + "g",
                                      tag=tagp + "g"))
                return (pool.tile([128, 256], F32, name=tagp + "if", tag=tagp + "if"),
                        pool.tile([128, 128], F32, name=tagp + "o", tag=tagp + "o"),
                        pool.tile([128, 128], F32, name=tagp + "g", tag=tagp + "g"))

            def gate_out(pgs, m):
                pg_if, pg_o, pg_g = pgs
                if m >= 12:
                    return pg_g[:, 32 * (m - 12):32 * (m - 12) + 32]
                if pg_o is None:
                    return pg_if[:, 32 * m:32 * m + 32]
                if m < 8:
                    return pg_if[:, 32 * m:32 * m + 32]
                return pg_o[:, 32 * (m - 8):32 * (m - 8) + 32]

            def gate_tail(pgs, c_st, hview, eng):
                """sig(i|f[|o]), tanh(g), c' = f*c + i*g, h = o*tanh(c')."""
                pg_if, pg_o, pg_g = pgs
                tg = ew.tile([128, 128], BF16, tag="tg")
                nc.scalar.activation(tg[:], pg_g[:], A.Tanh)
                nif = 384 if pg_o is None else 256
                sif = ew.tile([128, nif], BF16, tag="sif")
                nc.scalar.activation(sif[:], pg_if[:], A.Sigmoid)
                ig = ew.tile([128, 128], BF16, tag="ig")
                eng.tensor_mul(ig[:], sif[:, 0:128], tg[:])
                fc = ew.tile([128, 128], BF16, tag="fc")
                eng.tensor_mul(fc[:], sif[:, 128:256], c_st[:])
                eng.tensor_add(c_st[:], fc[:], ig[:])
                if pg_o is None:
                    so = sif[:, 256:384]
                else:
                    so_t = ew.tile([128, 128], BF16, tag="so")
                    nc.scalar.activation(so_t[:], pg_o[:], A.Sigmoid)
                    so = so_t[:]
                tc_ = ew.tile([128, 128], BF16, tag="tc")
                nc.scalar.activation(tc_[:], c_st[:], A.Tanh)
                eng.tensor_mul(hview, so, tc_[:])

            def l1_step(t):
                pgs = gate_slot(pg1p, "p1", False)
                xsrc = xTs[t // 4]
                xsl = slice((t % 4) * BSH, (t % 4 + 1) * BSH)
                for m in _M_ORDER:
                    osl = gate_out(pgs, m)
                    nc.tensor.matmul(osl, w1x_sb[0:DIN + 1, 128 * m:128 * (m + 1)],
                                     xsrc[0:DIN + 1, xsl],
                                     start=True, stop=(t == 0))
                    if t > 0:
                        for c in range(KC):
                            nc.tensor.matmul(
                                osl, wr1_sb[:, c, 128 * m:128 * (m + 1)],
                                h1T[:, (t - 1) % 4, c, :],
                                start=False, stop=(c == KC - 1))
                hv = h1T[:, t % 4, :, :]
                gate_tail(pgs, c1, hv, nc.vector)

            def l2_step(s):
                pgs = gate_slot(pg2p, "p2", False)
                # b2 seed: one K=16 one-hot matmul per gate tile
                nc.tensor.matmul(pgs[0][:], b2t_sb[:], oneh_sb[:, 0:256],
                                 start=True, stop=False, skip_group_check=True)
                nc.tensor.matmul(pgs[1][:], b2t_sb[:], oneh_sb[:, 256:384],
                                 start=True, stop=False, skip_group_check=True)
                nc.tensor.matmul(pgs[2][:], b2t_sb[:], oneh_sb[:, 384:512],
                                 start=True, stop=False, skip_group_check=True)
                for m in _M_ORDER:
                    osl = gate_out(pgs, m)
                    if s > 0:
                        for c in range(KC):
                            nc.tensor.matmul(
                                osl, wr2_sb[:, c, 128 * m:128 * (m + 1)],
                                h2T[:, c, s - 1, :],
                                start=False, stop=False, skip_group_check=True)
                    for c in range(KC):
                        nc.tensor.matmul(
                            osl, wk2_sb[:, c, 128 * m:128 * (m + 1)],
                            h1T[:, s % 4, c, :],
                            start=False, stop=(c == KC - 1),
                            skip_group_check=True)
                hv = bass.AP(tensor=h2T.tensor, offset=h2T.offset + s * BSH,
                             ap=[list(h2T.ap[0]), [T * BSH, KC], [1, BSH]])
                gate_tail(pgs, c2, hv, nc.vector)

            def head_slice(n):
                ps_head = psh.tile([33, TSL * BSH], F32, tag="hm")
                ps_mu = ps_head[0:1, :]
                ps_sg = ps_head[32:33, :]
                for c in range(KC):
                    rhs = h2T[:, c, n * TSL:(n + 1) * TSL, :]
                    nc.tensor.matmul(ps_mu, wms_sb[:, c, 0:1], rhs,
                                     start=(c == 0), stop=(c == KC - 1))
                    nc.tensor.matmul(ps_sg, wms_sb[:, c, 1:2], rhs,
                                     start=(c == 0), stop=(c == KC - 1))
                mu_sl = hew.tile([1, TSL * BSH], F32)
                nc.vector.tensor_scalar_add(mu_sl[:], ps_mu, bms_sb[0:1, 0:1])
                nc.vector.tensor_copy(
                    sgacc[0:1, TSL * BSH * n:TSL * BSH * (n + 1)], ps_sg)
                mu_view = bass.AP(tensor=mu_d[:].tensor, offset=n * TSL,
                                  ap=[[0, 1], [1, TSL], [T, BSH]])
                nc.sync.dma_start(out=mu_view, in_=mu_sl[:])

            for t in range(T + 2):
                if t % 4 == 0 and PRO + t // 4 < MT:
                    emit_tile(PRO + t // 4)
                if t < T:
                    l1_step(t)
                if t >= 2:
                    s = t - 2
                    l2_step(s)
                    if (s + 1) % TSL == 0:
                        head_slice((s + 1) // TSL - 1)

            # sigma = softplus(raw + bsig), batched once: Exp/Ln tables are in
            # a different act-func set than Sigmoid/Tanh, so doing this inside
            # the scan would cost two 1.3us table swaps every 16 steps
            nc.scalar.activation(exp_all[:], sgacc[:], A.Exp,
                                 bias=bms_sb[0:1, 1:2])
            nc.scalar.activation(sgacc[:], exp_all[:], A.Ln, bias=1.0)
            sg_view = bass.AP(
                tensor=sg_d[:].tensor, offset=0,
                ap=[[0, 1], [TSL, T // TSL], [1, TSL], [T, BSH]])
            nc.sync.dma_start(out=sg_view, in_=sgacc[:])

    return nc


def _marshal(inputs):
    """Host-side shard/layout marshalling (no compute beyond dtype cast/pad)."""
    bf = ml_dtypes.bfloat16
    xc = np.ascontiguousarray(np.asarray(inputs["x_cont"], np.float32))
    cat0 = np.asarray(inputs["cat0"]).astype(np.int32)
    cat1 = np.asarray(inputs["cat1"]).astype(np.int32)
    emb0 = np.asarray(inputs["emb0"], np.float32)
    emb1 = np.asarray(inputs["emb1"], np.float32)
    Wk1 = np.asarray(inputs["Wk1"], np.float32)
    Wr1 = np.asarray(inputs["Wr1"], np.float32)
    b1 = np.asarray(inputs["b1"], np.float32)
    Wk2 = np.asarray(inputs["Wk2"], np.float32)
    Wr2 = np.asarray(inputs["Wr2"], np.float32)
    b2 = np.asarray(inputs["b2"], np.float32)
    Wmu = np.asarray(inputs["Wmu"], np.float32)
    bmu = np.asarray(inputs["bmu"], np.float32)
    Wsig = np.asarray(inputs["Wsig"], np.float32)
    bsig = np.asarray(inputs["bsig"], np.float32)

    # permute gate columns [i, f, g, o] -> [i, f, o, g] so sigmoid gates are
    # contiguous in the transposed-gates free layout
    def perm(W):
        return np.concatenate(
            [W[..., 0:H], W[..., H:2 * H], W[..., 3 * H:4 * H],
             W[..., 2 * H:3 * H]], axis=-1)

    Wk1p, Wr1p, b1p = perm(Wk1), perm(Wr1), perm(b1)
    Wk2p, Wr2p, b2p = perm(Wk2), perm(Wr2), perm(b2)

    # x^T partition order: 0-31 emb0 dims, 32-47 emb1 dims, 48-55 x_cont, 56 ones
    w1x = np.zeros((64, G4), bf)
    w1x[0:E0, :] = Wk1p[F:F + E0, :].astype(bf)
    w1x[E0:E0 + E1, :] = Wk1p[F + E0:DIN, :].astype(bf)
    w1x[E0 + E1:DIN, :] = Wk1p[0:F, :].astype(bf)
    w1x[DIN, :] = b1p.astype(bf)

    wr1 = np.zeros((128, KC, G4), bf)
    wk2 = np.zeros((128, KC, G4), bf)
    wr2 = np.zeros((128, KC, G4), bf)
    wms = np.zeros((128, KC, 2), bf)
    for c in range(KC):
        wr1[:, c, :] = Wr1p[c * 128:(c + 1) * 128, :].astype(bf)
        wk2[:, c, :] = Wk2p[c * 128:(c + 1) * 128, :].astype(bf)
        wr2[:, c, :] = Wr2p[c * 128:(c + 1) * 128, :].astype(bf)
        wms[:, c, 0] = Wmu[c * 128:(c + 1) * 128, 0].astype(bf)
        wms[:, c, 1] = Wsig[c * 128:(c + 1) * 128, 0].astype(bf)
    b2t = np.ascontiguousarray(b2p.reshape(16, 128).astype(bf))
    oneh = np.kron(np.eye(16, dtype=np.float32),
                   np.ones((1, BSH), np.float32)).astype(bf)
    bms = np.array([[float(bmu.reshape(-1)[0]), float(bsig.reshape(-1)[0])]],
                   np.float32)

    MT = R // 128

    def wrap_idx(cat):  # [BSH, T] -> (t,b) rows -> [128, MT] int32
        lin = np.ascontiguousarray(cat.T).reshape(-1)       # (t, b) order
        return np.ascontiguousarray(lin.reshape(MT, 128).T.astype(np.int32))

    iota1 = np.arange(CARD1, dtype=np.float32).reshape(CARD1, 1)
    e1t16 = emb1.astype(bf)

    in_maps = []
    for cidx in range(NC_N):
        sl = slice(cidx * BSH, (cidx + 1) * BSH)
        xcs = xc[sl]                                        # [32, 192, 8]
        rows = xcs.transpose(1, 0, 2).reshape(R, F)      # (t,b) rows
        xcr = np.ascontiguousarray(
            rows.reshape(MT, 128, F).transpose(1, 0, 2).astype(np.float32))
        idx1r = np.ascontiguousarray(
            cat1[sl].T.reshape(1, R).astype(np.float32)).astype(bf)
        in_maps.append({
            "xcr": xcr,
            "idx0": wrap_idx(cat0[sl]),
            "idx1r": idx1r, "iota1": iota1,
            "e0tab": emb0, "e1t16": e1t16,
            "w1x": w1x, "wr1": wr1, "wk2": wk2, "wr2": wr2,
            "b2t": b2t, "oneh": oneh,
            "wms": wms, "bms": bms,
        })
    return in_maps


_RUN_KWARGS = {}   # test harness may set e.g. {"trace": True} for profiling
_LAST_RESULT = []


def kernel(**inputs):
    from concourse.bass_utils import run_bass_kernel_spmd
    in_maps = _marshal(inputs)
    nc = build_nc()
    res = run_bass_kernel_spmd(nc, in_maps, core_ids=list(range(NC_N)),
                               **_RUN_KWARGS)
    _LAST_RESULT.clear()
    _LAST_RESULT.append(res)
    mu = np.concatenate([r["mu"] for r in res.results], axis=0)      # [256, 192]
    sg = np.concatenate([r["sigma"] for r in res.results], axis=0)
    return (mu.reshape(B, T, 1).astype(np.float32),
            sg.reshape(B, T, 1).astype(np.float32))
